# revision 1
# baseline (speedup 1.0000x reference)
"""Causal MHA (B=4, L=2048, D=1024, H=16) on 8 NeuronCores.

Sharding: core c -> (batch b = c//2, head-group g = c%2). Data-parallel over
the 4 batches, tensor-parallel over heads (8 heads per core): wq/wk/wv
column-parallel, wo row-parallel. Each core returns a partial [L, D] output;
the host sums the two head-group partials per batch and adds wo_b.

Per-core device kernel (all matmuls fp32r: 1 cyc/row at N>=256, ~1.5e-4 rel):
  A) QT = (wq_g*0.125) @ q_b.T + bq  -> [512, 2048] SBUF (head dims on parts)
     KT likewise (unscaled).  V_aug = q_b @ wv_aug.T + vb -> [2048, 520] DRAM
     (per head: 64 dims + a ones column -> fused softmax denominator).
  B) per head h, per 512-wide q-slice: S.T[keys,q] = KT_h.T-slice @ QT_h
     (causal-trimmed N), exp on ScalarE, tri-mask on the diagonal 128-block,
     AV: psum[65, q] += V_aug_h[kb].T @ P.T  (row 64 = denominator).
     Normalize rows 0..63 by 1/denom (DVE recip + GpSimd partition_broadcast
     + DVE mul) -> ctxT [512, 2048] spilled to DRAM.
  C) out_partial[t, :] = sum_c ctxT[c, t-tile].T @ woT[c] -> [2048, 1024] f32.
"""

import numpy as np

import concourse.bacc as bacc
import concourse.bass as bass
import concourse.mybir as mybir
import concourse.tile as tile
from concourse.bass_utils import run_bass_kernel_spmd

F32 = mybir.dt.float32
F32R = mybir.dt.float32r

B, L, D, H, DK = 4, 2048, 1024, 16, 64
HD = 8            # heads per core
GW = 512          # head-group width (8 heads * 64)
AUGW = HD * (DK + 1)  # 520: per head 64 dims + ones col (ones LAST per head)
NCH = D // 128    # 8 contraction chunks
QS = 512          # q-slice width in attention
NQS = L // QS     # 4
NKB = L // 128    # 16 key blocks
NTT = L // 128    # 16 token tiles


def _r(ap):
    return ap


def _build_nc(dbg=False, phases="ABC"):
    nc = bacc.Bacc("TRN2", target_bir_lowering=False, debug=False, num_devices=8)

    xq = nc.dram_tensor("xq", [D, L], F32R, kind="ExternalInput").ap()
    xk = nc.dram_tensor("xk", [D, L], F32R, kind="ExternalInput").ap()
    xv = nc.dram_tensor("xv", [D, L], F32R, kind="ExternalInput").ap()
    wq = nc.dram_tensor("wq", [D, GW], F32R, kind="ExternalInput").ap()
    wk = nc.dram_tensor("wk", [D, GW], F32R, kind="ExternalInput").ap()
    wv = nc.dram_tensor("wv", [D, AUGW], F32R, kind="ExternalInput").ap()
    wo = nc.dram_tensor("wo", [GW, D], F32R, kind="ExternalInput").ap()
    bq = nc.dram_tensor("bq", [128, 4], F32, kind="ExternalInput").ap()
    bk = nc.dram_tensor("bk", [128, 4], F32, kind="ExternalInput").ap()
    vb = nc.dram_tensor("vb", [AUGW], F32, kind="ExternalInput").ap()
    msk = nc.dram_tensor("msk", [128, 128], F32, kind="ExternalInput").ap()
    outp = nc.dram_tensor("outp", [L, D], F32, kind="ExternalOutput").ap()
    if dbg:
        qt_dbg = nc.dram_tensor("qt_dbg", [128, 4 * L], F32, kind="ExternalOutput").ap()
        kt_dbg = nc.dram_tensor("kt_dbg", [128, 4 * L], F32, kind="ExternalOutput").ap()
        vg_dbg = nc.dram_tensor("vg_dbg", [L, AUGW], F32, kind="ExternalOutput").ap()
        ctx_dbg = nc.dram_tensor("ctx_dbg", [GW, L], F32, kind="ExternalOutput").ap()

    with tile.TileContext(nc) as tc:
        with (
            tc.tile_pool(name="persist", bufs=1) as persist,
            tc.tile_pool(name="xin", bufs=10) as xinp,
            tc.tile_pool(name="work", bufs=4) as workp,
            tc.tile_pool(name="pt", bufs=5) as ptp,
            tc.tile_pool(name="vh", bufs=2) as vhp,
            tc.tile_pool(name="small", bufs=4) as smallp,
            tc.tile_pool(name="outs", bufs=3) as outsp,
            tc.tile_pool(name="psA", bufs=6, space="PSUM") as psA,
            tc.tile_pool(name="psC", bufs=2, space="PSUM") as psC,
            tc.tile_pool(name="dram", bufs=1, space="DRAM") as dramp,
            tc.tile_pool(name="dnb", bufs=4, space="DRAM") as dnbp,
        ):
            # ---- persistent SBUF ----
            wq_s = persist.tile([128, NCH, GW], F32R, tag="wq")
            wk_s = persist.tile([128, NCH, GW], F32R, tag="wk")
            wv_s = persist.tile([128, NCH, AUGW], F32R, tag="wv")
            wo_s = persist.tile([128, 4, D], F32R, tag="wo")
            qt_s = persist.tile([128, 4, L], F32R, tag="qt")
            kt_s = persist.tile([128, 4, L], F32R, tag="kt")
            bq_s = persist.tile([128, 4], F32, tag="bq")
            bk_s = persist.tile([128, 4], F32, tag="bk")
            vb_s = persist.tile([128, AUGW], F32, tag="vb")
            msk_s = persist.tile([128, 128], F32, tag="msk")

            vg_d = dramp.tile([L, AUGW], F32R, tag="vg")
            ctx_d = dramp.tile([GW, L], F32R, tag="ctx")

            for c in range(NCH):
                nc.sync.dma_start(wq_s[:, c, :], wq[c * 128:(c + 1) * 128, :])
                nc.sync.dma_start(wk_s[:, c, :], wk[c * 128:(c + 1) * 128, :])
                nc.sync.dma_start(wv_s[:, c, :], wv[c * 128:(c + 1) * 128, :])
            for c in range(4):
                nc.sync.dma_start(wo_s[:, c, :], wo[c * 128:(c + 1) * 128, :])
            nc.sync.dma_start(bq_s[:, :], bq[:, :])
            nc.sync.dma_start(bk_s[:, :], bk[:, :])
            nc.sync.dma_start(msk_s[:, :], msk[:, :])
            vb_bcast = bass.AP(tensor=vb.tensor, offset=vb.offset,
                               ap=[[0, 128], [1, AUGW]])
            nc.gpsimd.dma_start(vb_s[:, :], vb_bcast)

            # ---- phase A: projections ----
            for n in range(4):  # 512-token slice
                for (src, w_s, dst, b_s) in ((xq, wq_s, qt_s, bq_s),
                                             (xk, wk_s, kt_s, bk_s)):
                    xt = []
                    for c in range(NCH):
                        t = xinp.tile([128, 512], F32R, tag="xin")
                        nc.sync.dma_start(
                            t[:, :], src[c * 128:(c + 1) * 128,
                                         n * 512:(n + 1) * 512])
                        xt.append(t)
                    pss = [psA.tile([128, 512], F32, tag="ps", name=f"psA{i}") for i in range(4)]
                    for c in range(NCH):
                        for m in range(4):
                            nc.tensor.matmul(
                                pss[m][:, :],
                                _r(w_s[:, c, m * 128:(m + 1) * 128]),
                                _r(xt[c][:, :]),
                                start=(c == 0), stop=(c == NCH - 1))
                    for m in range(4):
                        nc.vector.tensor_scalar_add(
                            dst[:, m, n * 512:(n + 1) * 512],
                            pss[m][:, :], b_s[:, m:m + 1])
                # V_aug
                xt = []
                for c in range(NCH):
                    t = xinp.tile([128, 512], F32R, tag="xin")
                    nc.sync.dma_start(
                        t[:, :], xv[c * 128:(c + 1) * 128,
                                    n * 512:(n + 1) * 512])
                    xt.append(t)
                for tt in range(4):  # token tile within slice
                    for hf in range(2):
                        ps = psA.tile([128, 260], F32, tag="ps")
                        for c in range(NCH):
                            nc.tensor.matmul(
                                ps[:, :],
                                _r(xt[c][:, tt * 128:(tt + 1) * 128]),
                                _r(wv_s[:, c, hf * 260:(hf + 1) * 260]),
                                start=(c == 0), stop=(c == NCH - 1))
                        vst = workp.tile([128, 260], F32R, tag="vst")
                        nc.vector.tensor_add(
                            vst[:, :], ps[:, :],
                            vb_s[:, hf * 260:(hf + 1) * 260])
                        nc.sync.dma_start(
                            vg_d[(n * 4 + tt) * 128:(n * 4 + tt + 1) * 128,
                                 hf * 260:(hf + 1) * 260],
                            vst[:, :])

            # ---- phase B: attention, two heads interleaved ----
            def emit_head_qs(h, vh, qs):
                po = (h % 2) * 64   # partition offset inside chunk
                mc = h // 2         # chunk index for this head
                cps = psC.tile([DK + 1, QS], F32, tag="cps", name=f"cps{h}_{qs}")
                nkb = 4 * qs + 4
                pts = [None] * nkb
                c0s = [None] * nkb

                def emit_st(kb):
                    col0 = max(0, kb * 128 - qs * QS)
                    sp = psA.tile([128, QS], F32, tag="ps", name=f"sp{h}_{qs}_{kb}")
                    nc.tensor.matmul(
                        sp[:, col0:],
                        _r(kt_s[po:po + 64, mc, kb * 128:(kb + 1) * 128]),
                        _r(qt_s[po:po + 64, mc,
                                qs * QS + col0:(qs + 1) * QS]),
                        start=True, stop=True)
                    pt = ptp.tile([128, QS], F32R, tag="pt", name=f"pt{h}_{qs}_{kb}")
                    nc.scalar.activation(
                        pt[:, col0:], sp[:, col0:],
                        func=mybir.ActivationFunctionType.Exp)
                    if col0 > 0 or kb == 4 * qs:
                        nc.vector.tensor_mul(
                            pt[:, col0:col0 + 128],
                            pt[:, col0:col0 + 128], msk_s[:, :])
                    pts[kb] = pt
                    c0s[kb] = col0

                def emit_av(kb):
                    col0 = c0s[kb]
                    nc.tensor.matmul(
                        cps[:, col0:],
                        _r(vh[:, kb, :]),
                        _r(pts[kb][:, col0:]),
                        start=(kb == 0), stop=(kb == nkb - 1))

                emit_st(0)
                emit_st(1)
                for kb in range(2, nkb):
                    emit_st(kb)
                    emit_av(kb - 2)
                emit_av(nkb - 2)
                emit_av(nkb - 1)

                rc = smallp.tile([128, QS], F32, tag="rc", name=f"rc{h}_{qs}")
                nc.vector.reciprocal(rc[64:65, :], cps[64:65, :])
                dn = dnbp.tile([1, QS], F32, tag="dn", name=f"dn{h}_{qs}")
                nc.sync.dma_start(dn[0:1, :], rc[64:65, :])
                bc = smallp.tile([64, QS], F32, tag="bc", name=f"bc{h}_{qs}")
                nc.sync.dma_start(bc[:, :],
                                  dn[0:1, :].partition_broadcast(64))
                co = workp.tile([64, QS], F32R, tag="co", name=f"co{h}_{qs}")
                nc.vector.tensor_mul(co[:, :], cps[0:64, :], bc[:, :])
                nc.sync.dma_start(
                    ctx_d[h * 64:(h + 1) * 64, qs * QS:(qs + 1) * QS],
                    co[:, :])

            for hp in (range(HD // 2) if "B" in phases else []):
                h0, h1 = 2 * hp, 2 * hp + 1
                vhs = []
                for h in (h0, h1):
                    vh = vhp.tile([128, NKB, DK + 1], F32R, tag="vh",
                                  name=f"vh{h}")
                    nc.sync.dma_start(
                        vh[:, :, :],
                        vg_d[:, h * 65:(h + 1) * 65].rearrange(
                            "(t p) a -> p t a", p=128))
                    vhs.append(vh)
                for qs in range(NQS):
                    emit_head_qs(h0, vhs[0], qs)
                    emit_head_qs(h1, vhs[1], qs)

            # ---- phase C: output projection ----
            for t in (range(NTT) if "C" in phases else []):
                cts = []
                for c in range(4):
                    ct = workp.tile([128, 128], F32R, tag="ct", bufs=8)
                    nc.sync.dma_start(
                        ct[:, :], ctx_d[c * 128:(c + 1) * 128,
                                        t * 128:(t + 1) * 128])
                    cts.append(ct)
                pss = [psA.tile([128, 512], F32, tag="ps", name=f"psC{i}") for i in range(2)]
                for c in range(4):
                    for n2 in range(2):
                        nc.tensor.matmul(
                            pss[n2][:, :], _r(cts[c][:, :]),
                            _r(wo_s[:, c, n2 * 512:(n2 + 1) * 512]),
                            start=(c == 0), stop=(c == 3))
                for n2 in range(2):
                    ot = outsp.tile([128, 512], F32, tag="ot")
                    nc.vector.tensor_copy(ot[:, :], pss[n2][:, :])
                    nc.sync.dma_start(
                        outp[t * 128:(t + 1) * 128,
                             n2 * 512:(n2 + 1) * 512], ot[:, :])

            if dbg:
                nc.sync.dma_start(qt_dbg[:, :], qt_s[:, :, :].bitcast(F32))
                nc.sync.dma_start(kt_dbg[:, :], kt_s[:, :, :].bitcast(F32))
                nc.sync.dma_start(vg_dbg[:, :], vg_d[:, :].bitcast(F32))
                nc.sync.dma_start(ctx_dbg[:, :], ctx_d[:, :].bitcast(F32))

    nc.compile()
    return nc


_NC = None
LAST_RESULTS = None


def kernel(**inputs):
    global _NC, LAST_RESULTS
    import os
    if _NC is None:
        _NC = _build_nc()

    f = lambda a: np.asarray(a, dtype=np.float32)
    q, k, v = f(inputs["q"]), f(inputs["k"]), f(inputs["v"])
    wq_w, wq_b = f(inputs["wq_w"]), f(inputs["wq_b"])
    wk_w, wk_b = f(inputs["wk_w"]), f(inputs["wk_b"])
    wv_w, wv_b = f(inputs["wv_w"]), f(inputs["wv_b"])
    wo_w, wo_b = f(inputs["wo_w"]), f(inputs["wo_b"])

    msk = np.ascontiguousarray(
        (np.arange(128)[None, :] >= np.arange(128)[:, None]).astype(np.float32))

    gmaps = []
    for g in range(2):
        sl = slice(g * GW, (g + 1) * GW)
        wqT = np.ascontiguousarray((wq_w[sl] * 0.125).T)
        wkT = np.ascontiguousarray(wk_w[sl].T)
        wvT = np.zeros((D, AUGW), np.float32)
        vbias = np.zeros((AUGW,), np.float32)
        for h in range(HD):
            wvT[:, h * 65:h * 65 + 64] = wv_w[g * GW + h * 64:
                                              g * GW + (h + 1) * 64].T
            vbias[h * 65:h * 65 + 64] = wv_b[g * GW + h * 64:
                                             g * GW + (h + 1) * 64]
            vbias[h * 65 + 64] = 1.0
        woT = np.ascontiguousarray(wo_w[:, sl].T)
        bqT = np.ascontiguousarray(
            (wq_b[sl] * 0.125).reshape(4, 128).T)
        bkT = np.ascontiguousarray(wk_b[sl].reshape(4, 128).T)
        gmaps.append(dict(wq=wqT, wk=wkT, wv=wvT, wo=woT, bq=bqT, bk=bkT,
                          vb=vbias, msk=msk))

    bmaps = []
    for b in range(B):
        bmaps.append(dict(
            xq=np.ascontiguousarray(q[b].T),
            xk=np.ascontiguousarray(k[b].T),
            xv=np.ascontiguousarray(v[b].T)))

    in_maps = [dict(**bmaps[c // 2], **gmaps[c % 2]) for c in range(8)]

    trace = bool(int(os.environ.get("KERNEL_TRACE", "0")))
    res = run_bass_kernel_spmd(_NC, in_maps, list(range(8)), trace=trace)
    LAST_RESULTS = res

    out = np.empty((B, L, D), np.float32)
    for b in range(B):
        out[b] = (res.results[2 * b]["outp"] + res.results[2 * b + 1]["outp"]
                  + wo_b[None, :])
    return out



# revision 57
# speedup vs baseline: 1.6073x; 1.6073x over previous
"""Causal MHA (B=4, L=2048, D=1024, H=16) on 8 NeuronCores.

Sharding: core c -> (batch b = c//2, head-group g = c%2). Data-parallel over
the 4 batches, tensor-parallel over heads (8 heads per core): wq/wk/wv
column-parallel, wo row-parallel. Each core returns a partial [L, D] output;
the host sums the two head-group partials per batch and adds wo_b.

v2: all-bf16, all-SBUF (no DRAM spills for V/ctx), chunky DMAs, phases
interleaved so the tile scheduler can fill PE gaps:
  A) QT/KT = w @ x + b -> SBUF [128, 4, 2048] bf16 (head dims on partitions),
     V_aug = x_tt @ wv_aug + vb -> vh_s [128 keypos, 16 kb, 520] bf16
     (per head 64 dims + ones column -> fused softmax denominator).
  B) per (head, 512-q-slice): S^T[keys, q] = KT_kb^T @ QT-slice into 2-bank
     psum pairs, exp on ACT (bf16 out), tri-mask muls on DVE, AV accumulates
     cps[65, q] (row 64 = denominator). recip (DVE) + partition_broadcast
     (GpSimd) + mul (DVE) -> stg[64, 8, 512]; 2 parity DMAs -> ctx_s.
  C) out[t, :] = sum_c ctx_s[:, c, t]^T @ wo_s[:, c, :] -> DVE copy -> DRAM.
"""

import numpy as np
import ml_dtypes

import concourse.bacc as bacc
import concourse.bass as bass
import concourse.mybir as mybir
import concourse.tile as tile
from concourse.bass_utils import run_bass_kernel_spmd

F32 = mybir.dt.float32
BF16 = mybir.dt.bfloat16

B, L, D, H, DK = 4, 2048, 1024, 16, 64
HD = 8                 # heads per core
GW = 512               # head-group width (8 heads * 64)
AUGW = HD * (DK + 1)   # 520: per head 64 dims + ones col (ones LAST per head)
NCH = D // 128         # 8 contraction chunks
QS = 512               # q-slice width in attention
NQS = L // QS          # 4
NKB = L // 128         # 16 key blocks
EXPF = mybir.ActivationFunctionType.Exp
IDF = mybir.ActivationFunctionType.Identity


def _build_nc(dbg=False):
    nc = bacc.Bacc("TRN2", target_bir_lowering=False, debug=False, num_devices=8)

    xq = nc.dram_tensor("xq", [D, L], BF16, kind="ExternalInput").ap()
    xk = nc.dram_tensor("xk", [D, L], BF16, kind="ExternalInput").ap()
    xv = nc.dram_tensor("xv", [D, L], BF16, kind="ExternalInput").ap()
    wq = nc.dram_tensor("wq", [D, GW], BF16, kind="ExternalInput").ap()
    wk = nc.dram_tensor("wk", [D, GW], BF16, kind="ExternalInput").ap()
    wv = nc.dram_tensor("wv", [D, AUGW], BF16, kind="ExternalInput").ap()
    wo = nc.dram_tensor("wo", [GW, D], BF16, kind="ExternalInput").ap()
    bq = nc.dram_tensor("bq", [128, 4], F32, kind="ExternalInput").ap()
    bk = nc.dram_tensor("bk", [128, 4], F32, kind="ExternalInput").ap()
    vb = nc.dram_tensor("vb", [128, AUGW], F32, kind="ExternalInput").ap()
    msk = nc.dram_tensor("msk", [128, 128], BF16, kind="ExternalInput").ap()
    outp = nc.dram_tensor("outp", [L, D], BF16, kind="ExternalOutput").ap()
    if dbg:
        qt_dbg = nc.dram_tensor("qt_dbg", [128, 4, L], BF16, kind="ExternalOutput").ap()
        kt_dbg = nc.dram_tensor("kt_dbg", [128, 4, L], BF16, kind="ExternalOutput").ap()
        vh_dbg = nc.dram_tensor("vh_dbg", [128, NKB, AUGW], BF16, kind="ExternalOutput").ap()
        ctx_dbg = nc.dram_tensor("ctx_dbg", [128, 4, L], BF16, kind="ExternalOutput").ap()

    with tile.TileContext(nc) as tc:
        with (
            tc.tile_pool(name="persist", bufs=1) as persist,
            tc.tile_pool(name="xin", bufs=6) as xinp,
            tc.tile_pool(name="ptA", bufs=2) as ptAp,
            tc.tile_pool(name="ptB", bufs=2) as ptBp,
            tc.tile_pool(name="stg", bufs=2) as stgp,
            tc.tile_pool(name="rc", bufs=2) as rcp,
            tc.tile_pool(name="bc", bufs=2) as bcp,
            tc.tile_pool(name="outs", bufs=2) as outsp,
            tc.tile_pool(name="pj", bufs=2, space="PSUM") as pjp,
            tc.tile_pool(name="spA", bufs=1, space="PSUM") as spAp,
            tc.tile_pool(name="spB", bufs=1, space="PSUM") as spBp,
            tc.tile_pool(name="cps", bufs=2, space="PSUM") as cpsp,
        ):
            # ---- persistent SBUF ----
            wq_s = persist.tile([128, NCH, GW], BF16, tag="wq")
            wk_s = persist.tile([128, NCH, GW], BF16, tag="wk")
            wv_s = persist.tile([128, NCH, AUGW], BF16, tag="wv")
            wo_s = persist.tile([128, 4, D], BF16, tag="wo")
            qt_s = persist.tile([128, 4, L], BF16, tag="qt")
            kt_s = persist.tile([128, 4, L], BF16, tag="kt")
            vh_s = persist.tile([128, NKB, AUGW], BF16, tag="vh")
            ctx_s = persist.tile([128, 4, L], BF16, tag="ctx")
            bq_s = persist.tile([128, 4], F32, tag="bq")
            bk_s = persist.tile([128, 4], F32, tag="bk")
            vb_s = persist.tile([128, AUGW], F32, tag="vb")
            msk_s = persist.tile([128, 128], BF16, tag="msk")

            SRCX = {"q": xq, "k": xk, "v": xv}

            def load_x(n, names=("q", "k", "v")):
                xt = {}
                for nm in names:
                    src = SRCX[nm]
                    t = xinp.tile([128, NCH, QS], BF16, tag="xin", name=f"x{nm}{n}")
                    nc.sync.dma_start(
                        t[:, :, :],
                        src[:, n * QS:(n + 1) * QS].rearrange(
                            "(c p) t -> p c t", p=128))
                    xt[nm] = t
                return xt

            # startup ordering: the q-projection of slice 0 only needs wq+
            # xq0, so those DMAs go first on the serialized DMA engine; the
            # slice-1 x loads are prefetched before wo so phase-A(1) filler
            # is ready when B(0) starts.
            nc.sync.dma_start(wq_s[:, :, :], wq.rearrange("(c p) n -> p c n", p=128))
            xt0 = load_x(0, names=("q",))
            nc.sync.dma_start(bq_s[:, :], bq[:, :])
            nc.sync.dma_start(wk_s[:, :, :], wk.rearrange("(c p) n -> p c n", p=128))
            xt0.update(load_x(0, names=("k",)))
            nc.sync.dma_start(bk_s[:, :], bk[:, :])
            nc.sync.dma_start(msk_s[:, :], msk[:, :])
            nc.sync.dma_start(wv_s[:, :, :], wv.rearrange("(c p) n -> p c n", p=128))
            xt0.update(load_x(0, names=("v",)))
            nc.sync.dma_start(vb_s[:, :], vb[:, :])

            # ---- phase A: projections for one 512-token slice ----
            # Returned as a list of unit-closures (one psum-tile each) so the
            # caller can interleave them with phase-C units in the shared pj
            # pool's rotation order.
            def A_units(n, xt):
                units = []

                def qk_unit(nm, w_s, dst, b_s, m):
                    ps = pjp.tile([128, QS], F32, tag="pj", name=f"pj{nm}{n}{m}")
                    for c in range(NCH):
                        nc.tensor.matmul(
                            ps[:, :],
                            w_s[:, c, m * 128:(m + 1) * 128],
                            xt[nm][:, c, :],
                            start=(c == 0), stop=(c == NCH - 1))
                    nc.vector.tensor_scalar_add(
                        dst[:, m, n * QS:(n + 1) * QS], ps[:, :],
                        b_s[:, m:m + 1])

                def v_unit(tt, hf):
                    ps = pjp.tile([128, 260], F32, tag="pj", name=f"pjv{n}{tt}{hf}")
                    for c in range(NCH):
                        nc.tensor.matmul(
                            ps[:, :],
                            xt["v"][:, c, tt * 128:(tt + 1) * 128],
                            wv_s[:, c, hf * 260:(hf + 1) * 260],
                            start=(c == 0), stop=(c == NCH - 1))
                    nc.vector.tensor_add(
                        vh_s[:, n * 4 + tt, hf * 260:(hf + 1) * 260],
                        ps[:, :], vb_s[:, hf * 260:(hf + 1) * 260])

                for (nm, w_s, dst, b_s) in (("q", wq_s, qt_s, bq_s),
                                            ("k", wk_s, kt_s, bk_s)):
                    for m in range(4):
                        units.append(lambda nm=nm, w=w_s, d=dst, b=b_s, m=m:
                                     qk_unit(nm, w, d, b, m))
                for tt in range(4):
                    for hf in range(2):
                        units.append(lambda tt=tt, hf=hf: v_unit(tt, hf))
                return units

            # ---- phase B: attention for one q-slice, two heads interleaved --
            def emit_head_pair(hp, qs, stg, mid=None):
                heads = (2 * hp, 2 * hp + 1)
                sps = (spAp, spBp)
                pts = (ptAp, ptBp)
                cps = {}
                for h in heads:
                    cps[h] = cpsp.tile([DK + 1, QS], F32, tag="cps",
                                       name=f"cps{h}_{qs}")
                npair = 2 * qs + 2
                pend = {h: [] for h in heads}  # (pt, c0, c1, kb0)

                def emit_s(h, i):
                    po = (h % 2) * 64
                    mc = h // 2
                    kb0, kb1 = 2 * i, 2 * i + 1
                    c0 = max(0, kb0 * 128 - qs * QS)
                    c1 = max(0, kb1 * 128 - qs * QS)
                    sp = sps[h % 2].tile([128, 2 * QS], F32, tag="sp",
                                         name=f"sp{h}_{qs}_{i}")
                    nc.tensor.matmul(
                        sp[:, c0:QS],
                        kt_s[po:po + 64, mc, kb0 * 128:(kb0 + 1) * 128],
                        qt_s[po:po + 64, mc, qs * QS + c0:(qs + 1) * QS],
                        start=True, stop=True)
                    nc.tensor.matmul(
                        sp[:, QS + c1:2 * QS],
                        kt_s[po:po + 64, mc, kb1 * 128:(kb1 + 1) * 128],
                        qt_s[po:po + 64, mc, qs * QS + c1:(qs + 1) * QS],
                        start=True, stop=True)
                    pt = pts[h % 2].tile([128, 2 * QS], BF16, tag="pt",
                                         name=f"pt{h}_{qs}_{i}")
                    if c1 <= 128:
                        # off-diag pair or first diagonal pair: one exp
                        # (spans the small stale gap [QS, QS+c1) harmlessly)
                        nc.scalar.activation(pt[:, c0:], sp[:, c0:], func=EXPF)
                    else:
                        nc.scalar.activation(pt[:, c0:QS], sp[:, c0:QS],
                                             func=EXPF)
                        nc.scalar.activation(pt[:, QS + c1:], sp[:, QS + c1:],
                                             func=EXPF)
                    if c0 > 0 or kb0 == 4 * qs:
                        nc.vector.tensor_mul(
                            pt[:, c0:c0 + 128], pt[:, c0:c0 + 128],
                            msk_s[:, :])
                    if c1 > 0 or kb1 == 4 * qs:
                        nc.vector.tensor_mul(
                            pt[:, QS + c1:QS + c1 + 128],
                            pt[:, QS + c1:QS + c1 + 128], msk_s[:, :])
                    pend[h].append((pt, c0, c1, kb0))

                def emit_av(h):
                    pt, c0, c1, kb0 = pend[h].pop(0)
                    kb1 = kb0 + 1
                    nc.tensor.matmul(
                        cps[h][:, c0:],
                        vh_s[:, kb0, h * 65:(h + 1) * 65],
                        pt[:, c0:QS],
                        start=(kb0 == 0), stop=False)
                    nc.tensor.matmul(
                        cps[h][:, c1:],
                        vh_s[:, kb1, h * 65:(h + 1) * 65],
                        pt[:, QS + c1:],
                        start=False, stop=(kb1 == 4 * qs + 3))

                for i in range(npair):
                    for h in heads:
                        emit_s(h, i)
                    if i > 0:
                        for h in heads:
                            emit_av(h)
                    if i == 1 and mid is not None:
                        mid()
                for h in heads:
                    emit_av(h)

                def normalize():
                    for h in heads:
                        # denominator recip (bf16) lives on partition 64;
                        # replicate to partitions 0:64 with a K=1 PE matmul
                        # against the mask's all-ones row (same partition),
                        # then stage through SBUF (ACT) for the DVE multiply.
                        rc = rcp.tile([DK + 1, QS], BF16, tag="rc",
                                      name=f"rc{h}_{qs}")
                        with nc.allow_low_precision(reason="bf16 denom"):
                            nc.vector.reciprocal(rc[64:65, :],
                                                 cps[h][64:65, :])
                        bc_ps = sps[h % 2].tile([64, QS], F32, tag="sp",
                                                name=f"bcp{h}_{qs}")
                        nc.tensor.matmul(bc_ps[:, :], msk_s[64:65, 64:128],
                                         rc[64:65, :], start=True, stop=True)
                        bcst = bcp.tile([64, QS], F32, tag="bc",
                                        name=f"bc{h}_{qs}")
                        nc.scalar.copy(bcst[:, :], bc_ps[:, :])
                        if h >= 4:
                            # parity-1 head: ctx partitions 64:128 via stg
                            nc.vector.tensor_mul(
                                stg[:, h - 4, :], cps[h][0:64, :], bcst[:, :])
                        else:
                            # parity-0 head: partitions align, write direct
                            nc.vector.tensor_mul(
                                ctx_s[0:64, h, qs * QS:(qs + 1) * QS],
                                cps[h][0:64, :], bcst[:, :])

                return normalize

            def emit_B(qs, pending=None):
                # parity-1 heads (4..7) first: their stg DMA overlaps the
                # parity-0 heads, whose normalize writes ctx_s directly.
                # Each pair's normalize is deferred into the NEXT pair's
                # attention loop (after its 2nd pair of exps) so the
                # recip->bcast->copy chain hides behind ready exp work.
                stg = stgp.tile([64, 4, QS], BF16, tag="stg", name=f"stg{qs}")
                state = {"n": pending}

                def mid():
                    if state["n"] is not None:
                        state["n"]()
                        state["n"] = None

                for hp in (2, 3, 0, 1):
                    nrm = emit_head_pair(hp, qs, stg, mid)
                    mid()  # in case the pair loop was too short
                    state["n"] = nrm
                    if hp == 0:
                        # normalizes of heads 4..7 have been emitted by now
                        nc.sync.dma_start(
                            ctx_s[64:128, :, qs * QS:(qs + 1) * QS],
                            stg[:, :, :])
                return state["n"]

            # ---- phase C: output projection for one q-slice of tokens ----
            # ---- phase C: output projection, unit = one (t-tile, 512-half) --
            def C_units(qs):
                units = []
                ots = {}

                def c_unit(t, n2):
                    if n2 == 0:
                        ots[t] = outsp.tile([128, D], BF16, tag="ot",
                                            name=f"ot{t}")
                    ot = ots[t]
                    ps = pcp_tile(t, n2)
                    for c in range(4):
                        nc.tensor.matmul(
                            ps[:, :],
                            ctx_s[:, c, t * 128:(t + 1) * 128],
                            wo_s[:, c, n2 * QS:(n2 + 1) * QS],
                            start=(c == 0), stop=(c == 3))
                    nc.vector.tensor_copy(
                        ot[:, n2 * QS:(n2 + 1) * QS], ps[:, :])
                    nc.sync.dma_start(
                        outp[t * 128:(t + 1) * 128, n2 * QS:(n2 + 1) * QS],
                        ot[:, n2 * QS:(n2 + 1) * QS])

                def pcp_tile(t, n2):
                    return pjp.tile([128, QS], F32, tag="pj",
                                    name=f"pc{t}{n2}")

                for j in range(4):
                    t = qs * 4 + j
                    for n2 in range(2):
                        units.append(lambda t=t, n2=n2: c_unit(t, n2))
                return units

            pending = None
            xts = {0: xt0, 1: load_x(1)}
            nc.sync.dma_start(wo_s[:, :, :], wo.rearrange("(c p) n -> p c n", p=128))
            for u in A_units(0, xt0):
                u()
            for qs in range(NQS):
                if qs + 2 < NQS:
                    xts[qs + 2] = load_x(qs + 2)
                pending = emit_B(qs, pending)
                if qs + 1 < NQS:
                    for u in A_units(qs + 1, xts[qs + 1]):
                        u()
            pending()
            # phase C emitted last: lowest scheduler priority, so its matmuls
            # act as opportunistic PE filler inside the ACT-bound B windows.
            for qs in range(NQS):
                for u in C_units(qs):
                    u()

            if dbg:
                nc.sync.dma_start(qt_dbg[:, :, :], qt_s[:, :, :])
                nc.sync.dma_start(kt_dbg[:, :, :], kt_s[:, :, :])
                nc.sync.dma_start(vh_dbg[:, :, :], vh_s[:, :, :])
                nc.sync.dma_start(ctx_dbg[:, :, :], ctx_s[:, :, :])

    nc.compile()
    return nc


_NC = None
LAST_RESULTS = None


def _bf16(a):
    return np.ascontiguousarray(a.astype(ml_dtypes.bfloat16))


def kernel(**inputs):
    global _NC, LAST_RESULTS
    import os
    if _NC is None:
        _NC = _build_nc(dbg=bool(int(os.environ.get("KERNEL_DBG", "0"))))

    f = lambda a: np.asarray(a, dtype=np.float32)
    q, k, v = f(inputs["q"]), f(inputs["k"]), f(inputs["v"])
    wq_w, wq_b = f(inputs["wq_w"]), f(inputs["wq_b"])
    wk_w, wk_b = f(inputs["wk_w"]), f(inputs["wk_b"])
    wv_w, wv_b = f(inputs["wv_w"]), f(inputs["wv_b"])
    wo_w, wo_b = f(inputs["wo_w"]), f(inputs["wo_b"])

    msk = np.ascontiguousarray(
        (np.arange(128)[None, :] >= np.arange(128)[:, None]).astype(np.float32))

    gmaps = []
    for g in range(2):
        sl = slice(g * GW, (g + 1) * GW)
        wqT = (wq_w[sl] * 0.125).T
        wkT = wk_w[sl].T
        wvT = np.zeros((D, AUGW), np.float32)
        vbias = np.zeros((AUGW,), np.float32)
        for h in range(HD):
            wvT[:, h * 65:h * 65 + 64] = wv_w[g * GW + h * 64:
                                              g * GW + (h + 1) * 64].T
            vbias[h * 65:h * 65 + 64] = wv_b[g * GW + h * 64:
                                             g * GW + (h + 1) * 64]
            vbias[h * 65 + 64] = 1.0
        # wo rows permuted: ctx chunk c, partition par*64+p0 <-> head par*4+c
        woT = np.zeros((GW, D), np.float32)
        for par in range(2):
            for c in range(4):
                h = par * 4 + c
                woT[c * 128 + par * 64:c * 128 + par * 64 + 64, :] = \
                    wo_w[:, g * GW + h * 64:g * GW + (h + 1) * 64].T
        bqT = np.ascontiguousarray((wq_b[sl] * 0.125).reshape(4, 128).T)
        bkT = np.ascontiguousarray(wk_b[sl].reshape(4, 128).T)
        vb_bc = np.broadcast_to(vbias[None, :], (128, AUGW))
        gmaps.append(dict(wq=_bf16(wqT), wk=_bf16(wkT), wv=_bf16(wvT),
                          wo=_bf16(woT), bq=bqT, bk=bkT,
                          vb=np.ascontiguousarray(vb_bc.astype(np.float32)),
                          msk=_bf16(msk)))

    bmaps = []
    for b in range(B):
        bmaps.append(dict(
            xq=_bf16(q[b].T),
            xk=_bf16(k[b].T),
            xv=_bf16(v[b].T)))

    in_maps = [dict(**bmaps[c // 2], **gmaps[c % 2]) for c in range(8)]

    trace = bool(int(os.environ.get("KERNEL_TRACE", "0")))
    res = run_bass_kernel_spmd(_NC, in_maps, list(range(8)), trace=trace)
    LAST_RESULTS = res

    out = np.empty((B, L, D), np.float32)
    for b in range(B):
        out[b] = (np.asarray(res.results[2 * b]["outp"], dtype=np.float32)
                  + np.asarray(res.results[2 * b + 1]["outp"], dtype=np.float32)
                  + wo_b[None, :])
    return out


# revision 62
# speedup vs baseline: 1.6130x; 1.0036x over previous
"""Causal MHA (B=4, L=2048, D=1024, H=16) on 8 NeuronCores.

Sharding: core c -> (batch b = c//2, head-group g = c%2). Data-parallel over
the 4 batches, tensor-parallel over heads (8 heads per core): wq/wk/wv
column-parallel, wo row-parallel. Each core returns a partial [L, D] output;
the host sums the two head-group partials per batch and adds wo_b.

v2: all-bf16, all-SBUF (no DRAM spills for V/ctx), chunky DMAs, phases
interleaved so the tile scheduler can fill PE gaps:
  A) QT/KT = w @ x + b -> SBUF [128, 4, 2048] bf16 (head dims on partitions),
     V_aug = x_tt @ wv_aug + vb -> vh_s [128 keypos, 16 kb, 520] bf16
     (per head 64 dims + ones column -> fused softmax denominator).
  B) per (head, 512-q-slice): S^T[keys, q] = KT_kb^T @ QT-slice into 2-bank
     psum pairs, exp on ACT (bf16 out), tri-mask muls on DVE, AV accumulates
     cps[65, q] (row 64 = denominator). recip (DVE) + partition_broadcast
     (GpSimd) + mul (DVE) -> stg[64, 8, 512]; 2 parity DMAs -> ctx_s.
  C) out[t, :] = sum_c ctx_s[:, c, t]^T @ wo_s[:, c, :] -> DVE copy -> DRAM.
"""

import numpy as np
import ml_dtypes

import concourse.bacc as bacc
import concourse.bass as bass
import concourse.mybir as mybir
import concourse.tile as tile
from concourse.bass_utils import run_bass_kernel_spmd

F32 = mybir.dt.float32
BF16 = mybir.dt.bfloat16

B, L, D, H, DK = 4, 2048, 1024, 16, 64
HD = 8                 # heads per core
GW = 512               # head-group width (8 heads * 64)
AUGW = HD * (DK + 1)   # 520: per head 64 dims + ones col (ones LAST per head)
NCH = D // 128         # 8 contraction chunks
QS = 512               # q-slice width in attention
NQS = L // QS          # 4
NKB = L // 128         # 16 key blocks
EXPF = mybir.ActivationFunctionType.Exp
IDF = mybir.ActivationFunctionType.Identity


def _build_nc(dbg=False):
    nc = bacc.Bacc("TRN2", target_bir_lowering=False, debug=False, num_devices=8)

    xq = nc.dram_tensor("xq", [D, L], BF16, kind="ExternalInput").ap()
    xk = nc.dram_tensor("xk", [D, L], BF16, kind="ExternalInput").ap()
    xv = nc.dram_tensor("xv", [D, L], BF16, kind="ExternalInput").ap()
    wq = nc.dram_tensor("wq", [D, GW], BF16, kind="ExternalInput").ap()
    wk = nc.dram_tensor("wk", [D, GW], BF16, kind="ExternalInput").ap()
    wv = nc.dram_tensor("wv", [D, AUGW], BF16, kind="ExternalInput").ap()
    wo = nc.dram_tensor("wo", [GW, D], BF16, kind="ExternalInput").ap()
    bq = nc.dram_tensor("bq", [128, 4], F32, kind="ExternalInput").ap()
    bk = nc.dram_tensor("bk", [128, 4], F32, kind="ExternalInput").ap()
    vb = nc.dram_tensor("vb", [128, AUGW], F32, kind="ExternalInput").ap()
    msk = nc.dram_tensor("msk", [128, 128], BF16, kind="ExternalInput").ap()
    outp = nc.dram_tensor("outp", [L, D], BF16, kind="ExternalOutput").ap()
    if dbg:
        qt_dbg = nc.dram_tensor("qt_dbg", [128, 4, L], BF16, kind="ExternalOutput").ap()
        kt_dbg = nc.dram_tensor("kt_dbg", [128, 4, L], BF16, kind="ExternalOutput").ap()
        vh_dbg = nc.dram_tensor("vh_dbg", [128, NKB, AUGW], BF16, kind="ExternalOutput").ap()
        ctx_dbg = nc.dram_tensor("ctx_dbg", [128, 4, L], BF16, kind="ExternalOutput").ap()

    with tile.TileContext(nc) as tc:
        with (
            tc.tile_pool(name="persist", bufs=1) as persist,
            tc.tile_pool(name="xin", bufs=8) as xinp,
            tc.tile_pool(name="ptA", bufs=3) as ptAp,
            tc.tile_pool(name="ptB", bufs=3) as ptBp,
            tc.tile_pool(name="stg", bufs=2) as stgp,
            tc.tile_pool(name="rc", bufs=3) as rcp,
            tc.tile_pool(name="bc", bufs=3) as bcp,
            tc.tile_pool(name="outs", bufs=3) as outsp,
            tc.tile_pool(name="pj", bufs=2, space="PSUM") as pjp,
            tc.tile_pool(name="spA", bufs=1, space="PSUM") as spAp,
            tc.tile_pool(name="spB", bufs=1, space="PSUM") as spBp,
            tc.tile_pool(name="cps", bufs=2, space="PSUM") as cpsp,
        ):
            # ---- persistent SBUF ----
            wq_s = persist.tile([128, NCH, GW], BF16, tag="wq")
            wk_s = persist.tile([128, NCH, GW], BF16, tag="wk")
            wv_s = persist.tile([128, NCH, AUGW], BF16, tag="wv")
            wo_s = persist.tile([128, 4, D], BF16, tag="wo")
            qt_s = persist.tile([128, 4, L], BF16, tag="qt")
            kt_s = persist.tile([128, 4, L], BF16, tag="kt")
            vh_s = persist.tile([128, NKB, AUGW], BF16, tag="vh")
            ctx_s = persist.tile([128, 4, L], BF16, tag="ctx")
            bq_s = persist.tile([128, 4], F32, tag="bq")
            bk_s = persist.tile([128, 4], F32, tag="bk")
            vb_s = persist.tile([128, AUGW], F32, tag="vb")
            msk_s = persist.tile([128, 128], BF16, tag="msk")

            SRCX = {"q": xq, "k": xk, "v": xv}

            def load_x(n, names=("q", "k", "v")):
                xt = {}
                for nm in names:
                    src = SRCX[nm]
                    t = xinp.tile([128, NCH, QS], BF16, tag="xin", name=f"x{nm}{n}")
                    nc.sync.dma_start(
                        t[:, :, :],
                        src[:, n * QS:(n + 1) * QS].rearrange(
                            "(c p) t -> p c t", p=128))
                    xt[nm] = t
                return xt

            # startup ordering: the q-projection of slice 0 only needs wq+
            # xq0, so those DMAs go first on the serialized DMA engine; the
            # slice-1 x loads are prefetched before wo so phase-A(1) filler
            # is ready when B(0) starts.
            nc.sync.dma_start(wq_s[:, :, :], wq.rearrange("(c p) n -> p c n", p=128))
            xt0 = load_x(0, names=("q",))
            nc.sync.dma_start(bq_s[:, :], bq[:, :])
            xt0.update(load_x(0, names=("k", "v")))
            nc.sync.dma_start(wk_s[:, :, :], wk.rearrange("(c p) n -> p c n", p=128))
            nc.sync.dma_start(bk_s[:, :], bk[:, :])
            nc.sync.dma_start(wv_s[:, :, :], wv.rearrange("(c p) n -> p c n", p=128))
            nc.sync.dma_start(vb_s[:, :], vb[:, :])
            nc.sync.dma_start(msk_s[:, :], msk[:, :])

            # ---- phase A: projections for one 512-token slice ----
            # Returned as a list of unit-closures (one psum-tile each) so the
            # caller can interleave them with phase-C units in the shared pj
            # pool's rotation order.
            def A_units(n, xt):
                units = []

                def qk_unit(nm, w_s, dst, b_s, m):
                    ps = pjp.tile([128, QS], F32, tag="pj", name=f"pj{nm}{n}{m}")
                    for c in range(NCH):
                        nc.tensor.matmul(
                            ps[:, :],
                            w_s[:, c, m * 128:(m + 1) * 128],
                            xt[nm][:, c, :],
                            start=(c == 0), stop=(c == NCH - 1))
                    nc.vector.tensor_scalar_add(
                        dst[:, m, n * QS:(n + 1) * QS], ps[:, :],
                        b_s[:, m:m + 1])

                def v_unit(tt, hf):
                    ps = pjp.tile([128, 260], F32, tag="pj", name=f"pjv{n}{tt}{hf}")
                    for c in range(NCH):
                        nc.tensor.matmul(
                            ps[:, :],
                            xt["v"][:, c, tt * 128:(tt + 1) * 128],
                            wv_s[:, c, hf * 260:(hf + 1) * 260],
                            start=(c == 0), stop=(c == NCH - 1))
                    nc.vector.tensor_add(
                        vh_s[:, n * 4 + tt, hf * 260:(hf + 1) * 260],
                        ps[:, :], vb_s[:, hf * 260:(hf + 1) * 260])

                for (nm, w_s, dst, b_s) in (("q", wq_s, qt_s, bq_s),
                                            ("k", wk_s, kt_s, bk_s)):
                    for m in range(4):
                        units.append(lambda nm=nm, w=w_s, d=dst, b=b_s, m=m:
                                     qk_unit(nm, w, d, b, m))
                for tt in range(4):
                    for hf in range(2):
                        units.append(lambda tt=tt, hf=hf: v_unit(tt, hf))
                return units

            # ---- phase B: attention for one q-slice, two heads interleaved --
            def emit_head_pair(hp, qs, stg, mid=None):
                heads = (2 * hp, 2 * hp + 1)
                sps = (spAp, spBp)
                pts = (ptAp, ptBp)
                cps = {}
                for h in heads:
                    cps[h] = cpsp.tile([DK + 1, QS], F32, tag="cps",
                                       name=f"cps{h}_{qs}")
                npair = 2 * qs + 2
                pend = {h: [] for h in heads}  # (pt, c0, c1, kb0)

                def emit_s(h, i):
                    po = (h % 2) * 64
                    mc = h // 2
                    kb0, kb1 = 2 * i, 2 * i + 1
                    c0 = max(0, kb0 * 128 - qs * QS)
                    c1 = max(0, kb1 * 128 - qs * QS)
                    sp = sps[h % 2].tile([128, 2 * QS], F32, tag="sp",
                                         name=f"sp{h}_{qs}_{i}")
                    nc.tensor.matmul(
                        sp[:, c0:QS],
                        kt_s[po:po + 64, mc, kb0 * 128:(kb0 + 1) * 128],
                        qt_s[po:po + 64, mc, qs * QS + c0:(qs + 1) * QS],
                        start=True, stop=True)
                    nc.tensor.matmul(
                        sp[:, QS + c1:2 * QS],
                        kt_s[po:po + 64, mc, kb1 * 128:(kb1 + 1) * 128],
                        qt_s[po:po + 64, mc, qs * QS + c1:(qs + 1) * QS],
                        start=True, stop=True)
                    pt = pts[h % 2].tile([128, 2 * QS], BF16, tag="pt",
                                         name=f"pt{h}_{qs}_{i}")
                    if c1 <= 128:
                        # off-diag pair or first diagonal pair: one exp
                        # (spans the small stale gap [QS, QS+c1) harmlessly)
                        nc.scalar.activation(pt[:, c0:], sp[:, c0:], func=EXPF)
                    else:
                        nc.scalar.activation(pt[:, c0:QS], sp[:, c0:QS],
                                             func=EXPF)
                        nc.scalar.activation(pt[:, QS + c1:], sp[:, QS + c1:],
                                             func=EXPF)
                    if c0 > 0 or kb0 == 4 * qs:
                        nc.vector.tensor_mul(
                            pt[:, c0:c0 + 128], pt[:, c0:c0 + 128],
                            msk_s[:, :])
                    if c1 > 0 or kb1 == 4 * qs:
                        nc.vector.tensor_mul(
                            pt[:, QS + c1:QS + c1 + 128],
                            pt[:, QS + c1:QS + c1 + 128], msk_s[:, :])
                    pend[h].append((pt, c0, c1, kb0))

                def emit_av(h):
                    pt, c0, c1, kb0 = pend[h].pop(0)
                    kb1 = kb0 + 1
                    nc.tensor.matmul(
                        cps[h][:, c0:],
                        vh_s[:, kb0, h * 65:(h + 1) * 65],
                        pt[:, c0:QS],
                        start=(kb0 == 0), stop=False)
                    nc.tensor.matmul(
                        cps[h][:, c1:],
                        vh_s[:, kb1, h * 65:(h + 1) * 65],
                        pt[:, QS + c1:],
                        start=False, stop=(kb1 == 4 * qs + 3))

                for i in range(npair):
                    for h in heads:
                        emit_s(h, i)
                    if i > 0:
                        for h in heads:
                            emit_av(h)
                    if i == 1 and mid is not None:
                        mid()
                for h in heads:
                    emit_av(h)

                def normalize():
                    for h in heads:
                        # denominator recip (bf16) lives on partition 64;
                        # replicate to partitions 0:64 with a K=1 PE matmul
                        # against the mask's all-ones row (same partition),
                        # then stage through SBUF (ACT) for the DVE multiply.
                        rc = rcp.tile([DK + 1, QS], BF16, tag="rc",
                                      name=f"rc{h}_{qs}")
                        with nc.allow_low_precision(reason="bf16 denom"):
                            nc.vector.reciprocal(rc[64:65, :],
                                                 cps[h][64:65, :])
                        bc_ps = sps[h % 2].tile([64, QS], F32, tag="sp",
                                                name=f"bcp{h}_{qs}")
                        nc.tensor.matmul(bc_ps[:, :], msk_s[64:65, 64:128],
                                         rc[64:65, :], start=True, stop=True)
                        bcst = bcp.tile([64, QS], F32, tag="bc",
                                        name=f"bc{h}_{qs}")
                        nc.scalar.copy(bcst[:, :], bc_ps[:, :])
                        if h >= 4:
                            # parity-1 head: ctx partitions 64:128 via stg
                            nc.vector.tensor_mul(
                                stg[:, h - 4, :], cps[h][0:64, :], bcst[:, :])
                        else:
                            # parity-0 head: partitions align, write direct
                            nc.vector.tensor_mul(
                                ctx_s[0:64, h, qs * QS:(qs + 1) * QS],
                                cps[h][0:64, :], bcst[:, :])

                return normalize

            def emit_B(qs, pending=None):
                # parity-1 heads (4..7) first: their stg DMA overlaps the
                # parity-0 heads, whose normalize writes ctx_s directly.
                # Each pair's normalize is deferred into the NEXT pair's
                # attention loop (after its 2nd pair of exps) so the
                # recip->bcast->copy chain hides behind ready exp work.
                stg = stgp.tile([64, 4, QS], BF16, tag="stg", name=f"stg{qs}")
                state = {"n": pending}

                def mid():
                    if state["n"] is not None:
                        state["n"]()
                        state["n"] = None

                for hp in (2, 3, 0, 1):
                    nrm = emit_head_pair(hp, qs, stg, mid)
                    mid()  # in case the pair loop was too short
                    state["n"] = nrm
                    if hp == 0:
                        # normalizes of heads 4..7 have been emitted by now
                        nc.sync.dma_start(
                            ctx_s[64:128, :, qs * QS:(qs + 1) * QS],
                            stg[:, :, :])
                return state["n"]

            # ---- phase C: output projection for one q-slice of tokens ----
            # ---- phase C: output projection, unit = one (t-tile, 512-half) --
            def C_units(qs):
                units = []
                ots = {}

                def c_unit(t, n2):
                    if n2 == 0:
                        ots[t] = outsp.tile([128, D], BF16, tag="ot",
                                            name=f"ot{t}")
                    ot = ots[t]
                    ps = pcp_tile(t, n2)
                    for c in range(4):
                        nc.tensor.matmul(
                            ps[:, :],
                            ctx_s[:, c, t * 128:(t + 1) * 128],
                            wo_s[:, c, n2 * QS:(n2 + 1) * QS],
                            start=(c == 0), stop=(c == 3))
                    nc.vector.tensor_copy(
                        ot[:, n2 * QS:(n2 + 1) * QS], ps[:, :])
                    nc.sync.dma_start(
                        outp[t * 128:(t + 1) * 128, n2 * QS:(n2 + 1) * QS],
                        ot[:, n2 * QS:(n2 + 1) * QS])

                def pcp_tile(t, n2):
                    return pjp.tile([128, QS], F32, tag="pj",
                                    name=f"pc{t}{n2}")

                for j in range(4):
                    t = qs * 4 + j
                    for n2 in range(2):
                        units.append(lambda t=t, n2=n2: c_unit(t, n2))
                return units

            pending = None
            xts = {0: xt0, 1: load_x(1)}
            nc.sync.dma_start(wo_s[:, :, :], wo.rearrange("(c p) n -> p c n", p=128))
            for u in A_units(0, xt0):
                u()
            for qs in range(NQS):
                if qs + 2 < NQS:
                    xts[qs + 2] = load_x(qs + 2)
                pending = emit_B(qs, pending)
                if qs + 1 < NQS:
                    for u in A_units(qs + 1, xts[qs + 1]):
                        u()
            pending()
            # phase C emitted last: lowest scheduler priority, so its matmuls
            # act as opportunistic PE filler inside the ACT-bound B windows.
            for qs in range(NQS):
                for u in C_units(qs):
                    u()

            if dbg:
                nc.sync.dma_start(qt_dbg[:, :, :], qt_s[:, :, :])
                nc.sync.dma_start(kt_dbg[:, :, :], kt_s[:, :, :])
                nc.sync.dma_start(vh_dbg[:, :, :], vh_s[:, :, :])
                nc.sync.dma_start(ctx_dbg[:, :, :], ctx_s[:, :, :])

    nc.compile()
    return nc


_NC = None
LAST_RESULTS = None


def _bf16(a):
    return np.ascontiguousarray(a.astype(ml_dtypes.bfloat16))


def kernel(**inputs):
    global _NC, LAST_RESULTS
    import os
    if _NC is None:
        _NC = _build_nc(dbg=bool(int(os.environ.get("KERNEL_DBG", "0"))))

    f = lambda a: np.asarray(a, dtype=np.float32)
    q, k, v = f(inputs["q"]), f(inputs["k"]), f(inputs["v"])
    wq_w, wq_b = f(inputs["wq_w"]), f(inputs["wq_b"])
    wk_w, wk_b = f(inputs["wk_w"]), f(inputs["wk_b"])
    wv_w, wv_b = f(inputs["wv_w"]), f(inputs["wv_b"])
    wo_w, wo_b = f(inputs["wo_w"]), f(inputs["wo_b"])

    msk = np.ascontiguousarray(
        (np.arange(128)[None, :] >= np.arange(128)[:, None]).astype(np.float32))

    gmaps = []
    for g in range(2):
        sl = slice(g * GW, (g + 1) * GW)
        wqT = (wq_w[sl] * 0.125).T
        wkT = wk_w[sl].T
        wvT = np.zeros((D, AUGW), np.float32)
        vbias = np.zeros((AUGW,), np.float32)
        for h in range(HD):
            wvT[:, h * 65:h * 65 + 64] = wv_w[g * GW + h * 64:
                                              g * GW + (h + 1) * 64].T
            vbias[h * 65:h * 65 + 64] = wv_b[g * GW + h * 64:
                                             g * GW + (h + 1) * 64]
            vbias[h * 65 + 64] = 1.0
        # wo rows permuted: ctx chunk c, partition par*64+p0 <-> head par*4+c
        woT = np.zeros((GW, D), np.float32)
        for par in range(2):
            for c in range(4):
                h = par * 4 + c
                woT[c * 128 + par * 64:c * 128 + par * 64 + 64, :] = \
                    wo_w[:, g * GW + h * 64:g * GW + (h + 1) * 64].T
        bqT = np.ascontiguousarray((wq_b[sl] * 0.125).reshape(4, 128).T)
        bkT = np.ascontiguousarray(wk_b[sl].reshape(4, 128).T)
        vb_bc = np.broadcast_to(vbias[None, :], (128, AUGW))
        gmaps.append(dict(wq=_bf16(wqT), wk=_bf16(wkT), wv=_bf16(wvT),
                          wo=_bf16(woT), bq=bqT, bk=bkT,
                          vb=np.ascontiguousarray(vb_bc.astype(np.float32)),
                          msk=_bf16(msk)))

    bmaps = []
    for b in range(B):
        bmaps.append(dict(
            xq=_bf16(q[b].T),
            xk=_bf16(k[b].T),
            xv=_bf16(v[b].T)))

    in_maps = [dict(**bmaps[c // 2], **gmaps[c % 2]) for c in range(8)]

    trace = bool(int(os.environ.get("KERNEL_TRACE", "0")))
    res = run_bass_kernel_spmd(_NC, in_maps, list(range(8)), trace=trace)
    LAST_RESULTS = res

    out = np.empty((B, L, D), np.float32)
    for b in range(B):
        out[b] = (np.asarray(res.results[2 * b]["outp"], dtype=np.float32)
                  + np.asarray(res.results[2 * b + 1]["outp"], dtype=np.float32)
                  + wo_b[None, :])
    return out


# revision 71
# speedup vs baseline: 1.6480x; 1.0217x over previous
"""Causal MHA (B=4, L=2048, D=1024, H=16) on 8 NeuronCores.

Sharding: core c -> (batch b = c//2, head-group g = c%2). Data-parallel over
the 4 batches, tensor-parallel over heads (8 heads per core): wq/wk/wv
column-parallel, wo row-parallel. Each core returns a partial [L, D] output;
the host sums the two head-group partials per batch and adds wo_b.

v2: all-bf16, all-SBUF (no DRAM spills for V/ctx), chunky DMAs, phases
interleaved so the tile scheduler can fill PE gaps (419.5us -> 260.1us):
  A) QT/KT = w @ x + b -> SBUF [128, 4, 2048] bf16 (head dims on partitions),
     V_aug = x_tt @ wv_aug + vb -> vh_s [128 keypos, 16 kb, 520] bf16
     (per head 64 dims + ones column -> fused softmax denominator).
     Emitted as per-psum-tile units so A(qs+1) rides as PE filler inside the
     ACT-bound B(qs) window.
  B) per (head, 512-q-slice): S^T[keys, q] = KT_kb^T @ QT-slice, kb-pairs
     into one 2-bank psum tile, one exp per pair on ACT (bf16 out, exact
     causal trims), tri-mask muls on DVE, AV accumulates cps[65, q]
     (row 64 = denominator).  Normalize: recip (DVE, bf16, partition 64) ->
     K=1 PE matmul against the mask's all-ones row broadcasts it to
     partitions 0:64 -> ACT copy to SBUF -> DVE mul.  Each pair's normalize
     is deferred into the next head-pair's loop so its cross-engine chain
     hides behind ready exp work.  Parity-0 heads write ctx_s directly
     (partitions align); parity-1 heads stage + one DMA per q-slice.
  C) out[t, :] = sum_c ctx_s[:, c, t]^T @ wo_s[:, c, :] -> DVE copy -> DRAM
     (bf16); emitted last so its matmuls are lowest-priority PE filler.
"""

import numpy as np
import ml_dtypes

import concourse.bacc as bacc
import concourse.bass as bass
import concourse.mybir as mybir
import concourse.tile as tile
from concourse.bass_utils import run_bass_kernel_spmd

F32 = mybir.dt.float32
BF16 = mybir.dt.bfloat16

B, L, D, H, DK = 4, 2048, 1024, 16, 64
HD = 8                 # heads per core
GW = 512               # head-group width (8 heads * 64)
AUGW = HD * (DK + 1)   # 520: per head 64 dims + ones col (ones LAST per head)
NCH = D // 128         # 8 contraction chunks
QS = 512               # q-slice width in attention
NQS = L // QS          # 4
NKB = L // 128         # 16 key blocks
EXPF = mybir.ActivationFunctionType.Exp
IDF = mybir.ActivationFunctionType.Identity


def _build_nc(dbg=False):
    nc = bacc.Bacc("TRN2", target_bir_lowering=False, debug=False, num_devices=8)

    xq = nc.dram_tensor("xq", [D, L], BF16, kind="ExternalInput").ap()
    xk = nc.dram_tensor("xk", [D, L], BF16, kind="ExternalInput").ap()
    xv = nc.dram_tensor("xv", [D, L], BF16, kind="ExternalInput").ap()
    wq = nc.dram_tensor("wq", [D, GW], BF16, kind="ExternalInput").ap()
    wk = nc.dram_tensor("wk", [D, GW], BF16, kind="ExternalInput").ap()
    wv = nc.dram_tensor("wv", [D, AUGW], BF16, kind="ExternalInput").ap()
    wo = nc.dram_tensor("wo", [GW, D], BF16, kind="ExternalInput").ap()
    bq = nc.dram_tensor("bq", [128, 4], F32, kind="ExternalInput").ap()
    bk = nc.dram_tensor("bk", [128, 4], F32, kind="ExternalInput").ap()
    vb = nc.dram_tensor("vb", [128, AUGW], F32, kind="ExternalInput").ap()
    msk = nc.dram_tensor("msk", [128, 128], BF16, kind="ExternalInput").ap()
    outp = nc.dram_tensor("outp", [L, D], BF16, kind="ExternalOutput").ap()
    if dbg:
        qt_dbg = nc.dram_tensor("qt_dbg", [128, 4, L], BF16, kind="ExternalOutput").ap()
        kt_dbg = nc.dram_tensor("kt_dbg", [128, 4, L], BF16, kind="ExternalOutput").ap()
        vh_dbg = nc.dram_tensor("vh_dbg", [128, NKB, AUGW], BF16, kind="ExternalOutput").ap()
        ctx_dbg = nc.dram_tensor("ctx_dbg", [128, 4, L], BF16, kind="ExternalOutput").ap()

    with tile.TileContext(nc) as tc:
        with (
            tc.tile_pool(name="persist", bufs=1) as persist,
            tc.tile_pool(name="xin", bufs=8) as xinp,
            tc.tile_pool(name="ptA", bufs=3) as ptAp,
            tc.tile_pool(name="ptB", bufs=3) as ptBp,
            tc.tile_pool(name="stg", bufs=2) as stgp,
            tc.tile_pool(name="rc", bufs=3) as rcp,
            tc.tile_pool(name="bc", bufs=3) as bcp,
            tc.tile_pool(name="outs", bufs=3) as outsp,
            tc.tile_pool(name="pj", bufs=2, space="PSUM") as pjp,
            tc.tile_pool(name="spA", bufs=1, space="PSUM") as spAp,
            tc.tile_pool(name="spB", bufs=1, space="PSUM") as spBp,
            tc.tile_pool(name="cps", bufs=2, space="PSUM") as cpsp,
        ):
            # ---- persistent SBUF ----
            wq_s = persist.tile([128, NCH, GW], BF16, tag="wq")
            wk_s = persist.tile([128, NCH, GW], BF16, tag="wk")
            wv_s = persist.tile([128, NCH, AUGW], BF16, tag="wv")
            wo_s = persist.tile([128, 4, D], BF16, tag="wo")
            qt_s = persist.tile([128, 4, L], BF16, tag="qt")
            kt_s = persist.tile([128, 4, L], BF16, tag="kt")
            vh_s = persist.tile([128, NKB, AUGW], BF16, tag="vh")
            ctx_s = persist.tile([128, 4, L], BF16, tag="ctx")
            bq_s = persist.tile([128, 4], F32, tag="bq")
            bk_s = persist.tile([128, 4], F32, tag="bk")
            vb_s = persist.tile([128, AUGW], F32, tag="vb")
            msk_s = persist.tile([128, 128], BF16, tag="msk")

            SRCX = {"q": xq, "k": xk, "v": xv}

            def load_x(n, names=("q", "k", "v")):
                xt = {}
                for nm in names:
                    src = SRCX[nm]
                    t = xinp.tile([128, NCH, QS], BF16, tag="xin", name=f"x{nm}{n}")
                    nc.sync.dma_start(
                        t[:, :, :],
                        src[:, n * QS:(n + 1) * QS].rearrange(
                            "(c p) t -> p c t", p=128))
                    xt[nm] = t
                return xt

            # startup ordering: the q-projection of slice 0 only needs wq+
            # xq0, so those DMAs go first on the serialized DMA engine; the
            # slice-1 x loads are prefetched before wo so phase-A(1) filler
            # is ready when B(0) starts.
            # wq/xq0 in chunk-halves: the first four contraction chunks of
            # the q-projection can start while the second halves stream in.
            xt0 = {"q": xinp.tile([128, NCH, QS], BF16, tag="xin", name="xq0")}
            for c4 in range(4):
                c0, c1 = c4 * 2, c4 * 2 + 2
                nc.sync.dma_start(
                    wq_s[:, c0:c1, :],
                    wq[c0 * 128:c1 * 128, :].rearrange("(c p) n -> p c n", p=128))
                nc.sync.dma_start(
                    xt0["q"][:, c0:c1, :],
                    xq[c0 * 128:c1 * 128, 0:QS].rearrange("(c p) t -> p c t", p=128))
            nc.sync.dma_start(bq_s[:, :], bq[:, :])
            xt0["k"] = xinp.tile([128, NCH, QS], BF16, tag="xin", name="xk0")
            nc.sync.dma_start(wk_s[:, 0:4, :],
                              wk[0:512, :].rearrange("(c p) n -> p c n", p=128))
            nc.sync.dma_start(xt0["k"][:, 0:4, :],
                              xk[0:512, 0:QS].rearrange("(c p) t -> p c t", p=128))
            nc.sync.dma_start(wk_s[:, 4:8, :],
                              wk[512:1024, :].rearrange("(c p) n -> p c n", p=128))
            nc.sync.dma_start(xt0["k"][:, 4:8, :],
                              xk[512:1024, 0:QS].rearrange("(c p) t -> p c t", p=128))
            nc.sync.dma_start(bk_s[:, :], bk[:, :])
            xt0.update(load_x(0, names=("v",)))
            nc.sync.dma_start(wv_s[:, :, :], wv.rearrange("(c p) n -> p c n", p=128))
            nc.sync.dma_start(vb_s[:, :], vb[:, :])
            nc.sync.dma_start(msk_s[:, :], msk[:, :])

            # ---- phase A: projections for one 512-token slice ----
            # Returned as a list of unit-closures (one psum-tile each) so the
            # caller can interleave them with phase-C units in the shared pj
            # pool's rotation order.
            def A_units(n, xt):
                units = []

                def qk_unit(nm, w_s, dst, b_s, m):
                    ps = pjp.tile([128, QS], F32, tag="pj", name=f"pj{nm}{n}{m}")
                    for c in range(NCH):
                        nc.tensor.matmul(
                            ps[:, :],
                            w_s[:, c, m * 128:(m + 1) * 128],
                            xt[nm][:, c, :],
                            start=(c == 0), stop=(c == NCH - 1))
                    nc.vector.tensor_scalar_add(
                        dst[:, m, n * QS:(n + 1) * QS], ps[:, :],
                        b_s[:, m:m + 1])

                def v_unit(tt, hf):
                    ps = pjp.tile([128, 260], F32, tag="pj", name=f"pjv{n}{tt}{hf}")
                    for c in range(NCH):
                        nc.tensor.matmul(
                            ps[:, :],
                            xt["v"][:, c, tt * 128:(tt + 1) * 128],
                            wv_s[:, c, hf * 260:(hf + 1) * 260],
                            start=(c == 0), stop=(c == NCH - 1))
                    nc.vector.tensor_add(
                        vh_s[:, n * 4 + tt, hf * 260:(hf + 1) * 260],
                        ps[:, :], vb_s[:, hf * 260:(hf + 1) * 260])

                for (nm, w_s, dst, b_s) in (("q", wq_s, qt_s, bq_s),
                                            ("k", wk_s, kt_s, bk_s)):
                    for m in range(4):
                        units.append(lambda nm=nm, w=w_s, d=dst, b=b_s, m=m:
                                     qk_unit(nm, w, d, b, m))
                for tt in range(4):
                    for hf in range(2):
                        units.append(lambda tt=tt, hf=hf: v_unit(tt, hf))
                return units

            # ---- phase B: attention for one q-slice, two heads interleaved --
            def emit_head_pair(hp, qs, stg, mid=None):
                heads = (2 * hp, 2 * hp + 1)
                sps = (spAp, spBp)
                pts = (ptAp, ptBp)
                cps = {}
                for h in heads:
                    cps[h] = cpsp.tile([DK + 1, QS], F32, tag="cps",
                                       name=f"cps{h}_{qs}")
                npair = 2 * qs + 2
                pend = {h: [] for h in heads}  # (pt, c0, c1, kb0)

                def emit_s(h, i):
                    po = (h % 2) * 64
                    mc = h // 2
                    kb0, kb1 = 2 * i, 2 * i + 1
                    c0 = max(0, kb0 * 128 - qs * QS)
                    c1 = max(0, kb1 * 128 - qs * QS)
                    sp = sps[h % 2].tile([128, 2 * QS], F32, tag="sp",
                                         name=f"sp{h}_{qs}_{i}")
                    nc.tensor.matmul(
                        sp[:, c0:QS],
                        kt_s[po:po + 64, mc, kb0 * 128:(kb0 + 1) * 128],
                        qt_s[po:po + 64, mc, qs * QS + c0:(qs + 1) * QS],
                        start=True, stop=True)
                    nc.tensor.matmul(
                        sp[:, QS + c1:2 * QS],
                        kt_s[po:po + 64, mc, kb1 * 128:(kb1 + 1) * 128],
                        qt_s[po:po + 64, mc, qs * QS + c1:(qs + 1) * QS],
                        start=True, stop=True)
                    pt = pts[h % 2].tile([128, 2 * QS], BF16, tag="pt",
                                         name=f"pt{h}_{qs}_{i}")
                    if c1 <= 128:
                        # off-diag pair or first diagonal pair: one exp
                        # (spans the small stale gap [QS, QS+c1) harmlessly)
                        nc.scalar.activation(pt[:, c0:], sp[:, c0:], func=EXPF)
                    else:
                        nc.scalar.activation(pt[:, c0:QS], sp[:, c0:QS],
                                             func=EXPF)
                        nc.scalar.activation(pt[:, QS + c1:], sp[:, QS + c1:],
                                             func=EXPF)
                    if c0 > 0 or kb0 == 4 * qs:
                        nc.vector.tensor_mul(
                            pt[:, c0:c0 + 128], pt[:, c0:c0 + 128],
                            msk_s[:, :])
                    if c1 > 0 or kb1 == 4 * qs:
                        nc.vector.tensor_mul(
                            pt[:, QS + c1:QS + c1 + 128],
                            pt[:, QS + c1:QS + c1 + 128], msk_s[:, :])
                    pend[h].append((pt, c0, c1, kb0))

                def emit_av(h):
                    pt, c0, c1, kb0 = pend[h].pop(0)
                    kb1 = kb0 + 1
                    nc.tensor.matmul(
                        cps[h][:, c0:],
                        vh_s[:, kb0, h * 65:(h + 1) * 65],
                        pt[:, c0:QS],
                        start=(kb0 == 0), stop=False)
                    nc.tensor.matmul(
                        cps[h][:, c1:],
                        vh_s[:, kb1, h * 65:(h + 1) * 65],
                        pt[:, QS + c1:],
                        start=False, stop=(kb1 == 4 * qs + 3))

                for i in range(npair):
                    for h in heads:
                        emit_s(h, i)
                    if i > 0:
                        for h in heads:
                            emit_av(h)
                    if i == 1 and mid is not None:
                        mid()
                for h in heads:
                    emit_av(h)

                def normalize():
                    for h in heads:
                        # denominator recip (bf16) lives on partition 64;
                        # replicate to partitions 0:64 with a K=1 PE matmul
                        # against the mask's all-ones row (same partition),
                        # then stage through SBUF (ACT) for the DVE multiply.
                        rc = rcp.tile([DK + 1, QS], BF16, tag="rc",
                                      name=f"rc{h}_{qs}")
                        with nc.allow_low_precision(reason="bf16 denom"):
                            nc.vector.reciprocal(rc[64:65, :],
                                                 cps[h][64:65, :])
                        bc_ps = sps[h % 2].tile([64, QS], F32, tag="sp",
                                                name=f"bcp{h}_{qs}")
                        nc.tensor.matmul(bc_ps[:, :], msk_s[64:65, 64:128],
                                         rc[64:65, :], start=True, stop=True)
                        bcst = bcp.tile([64, QS], F32, tag="bc",
                                        name=f"bc{h}_{qs}")
                        nc.scalar.copy(bcst[:, :], bc_ps[:, :])
                        if h >= 4:
                            # parity-1 head: ctx partitions 64:128 via stg
                            nc.vector.tensor_mul(
                                stg[:, h - 4, :], cps[h][0:64, :], bcst[:, :])
                        else:
                            # parity-0 head: partitions align, write direct
                            nc.vector.tensor_mul(
                                ctx_s[0:64, h, qs * QS:(qs + 1) * QS],
                                cps[h][0:64, :], bcst[:, :])

                return normalize

            def emit_B(qs, pending=None):
                # parity-1 heads (4..7) first: their stg DMA overlaps the
                # parity-0 heads, whose normalize writes ctx_s directly.
                # Each pair's normalize is deferred into the NEXT pair's
                # attention loop (after its 2nd pair of exps) so the
                # recip->bcast->copy chain hides behind ready exp work.
                stg = stgp.tile([64, 4, QS], BF16, tag="stg", name=f"stg{qs}")
                state = {"n": pending}

                def mid():
                    if state["n"] is not None:
                        state["n"]()
                        state["n"] = None

                for hp in (2, 3, 0, 1):
                    nrm = emit_head_pair(hp, qs, stg, mid)
                    mid()  # in case the pair loop was too short
                    state["n"] = nrm
                    if hp == 0:
                        # normalizes of heads 4..7 have been emitted by now
                        nc.sync.dma_start(
                            ctx_s[64:128, :, qs * QS:(qs + 1) * QS],
                            stg[:, :, :])
                return state["n"]

            # ---- phase C: output projection for one q-slice of tokens ----
            # ---- phase C: output projection, unit = one (t-tile, 512-half) --
            def C_units(qs):
                units = []
                ots = {}

                def c_unit(t, n2):
                    if n2 == 0:
                        ots[t] = outsp.tile([128, D], BF16, tag="ot",
                                            name=f"ot{t}")
                    ot = ots[t]
                    ps = pcp_tile(t, n2)
                    for c in range(4):
                        nc.tensor.matmul(
                            ps[:, :],
                            ctx_s[:, c, t * 128:(t + 1) * 128],
                            wo_s[:, c, n2 * QS:(n2 + 1) * QS],
                            start=(c == 0), stop=(c == 3))
                    nc.vector.tensor_copy(
                        ot[:, n2 * QS:(n2 + 1) * QS], ps[:, :])
                    nc.sync.dma_start(
                        outp[t * 128:(t + 1) * 128, n2 * QS:(n2 + 1) * QS],
                        ot[:, n2 * QS:(n2 + 1) * QS])

                def pcp_tile(t, n2):
                    return pjp.tile([128, QS], F32, tag="pj",
                                    name=f"pc{t}{n2}")

                for j in range(4):
                    t = qs * 4 + j
                    for n2 in range(2):
                        units.append(lambda t=t, n2=n2: c_unit(t, n2))
                return units

            pending = None
            xts = {0: xt0, 1: load_x(1)}
            nc.sync.dma_start(wo_s[:, :, :], wo.rearrange("(c p) n -> p c n", p=128))
            for u in A_units(0, xt0):
                u()
            for qs in range(NQS):
                if qs + 2 < NQS:
                    xts[qs + 2] = load_x(qs + 2)
                pending = emit_B(qs, pending)
                if qs + 1 < NQS:
                    for u in A_units(qs + 1, xts[qs + 1]):
                        u()
            pending()
            # phase C emitted last: lowest scheduler priority, so its matmuls
            # act as opportunistic PE filler inside the ACT-bound B windows.
            for qs in range(NQS):
                for u in C_units(qs):
                    u()

            if dbg:
                nc.sync.dma_start(qt_dbg[:, :, :], qt_s[:, :, :])
                nc.sync.dma_start(kt_dbg[:, :, :], kt_s[:, :, :])
                nc.sync.dma_start(vh_dbg[:, :, :], vh_s[:, :, :])
                nc.sync.dma_start(ctx_dbg[:, :, :], ctx_s[:, :, :])

    nc.compile()
    return nc


_NC = None
LAST_RESULTS = None


def _bf16(a):
    return np.ascontiguousarray(a.astype(ml_dtypes.bfloat16))


def kernel(**inputs):
    global _NC, LAST_RESULTS
    import os
    if _NC is None:
        _NC = _build_nc(dbg=bool(int(os.environ.get("KERNEL_DBG", "0"))))

    f = lambda a: np.asarray(a, dtype=np.float32)
    q, k, v = f(inputs["q"]), f(inputs["k"]), f(inputs["v"])
    wq_w, wq_b = f(inputs["wq_w"]), f(inputs["wq_b"])
    wk_w, wk_b = f(inputs["wk_w"]), f(inputs["wk_b"])
    wv_w, wv_b = f(inputs["wv_w"]), f(inputs["wv_b"])
    wo_w, wo_b = f(inputs["wo_w"]), f(inputs["wo_b"])

    msk = np.ascontiguousarray(
        (np.arange(128)[None, :] >= np.arange(128)[:, None]).astype(np.float32))

    gmaps = []
    for g in range(2):
        sl = slice(g * GW, (g + 1) * GW)
        wqT = (wq_w[sl] * 0.125).T
        wkT = wk_w[sl].T
        wvT = np.zeros((D, AUGW), np.float32)
        vbias = np.zeros((AUGW,), np.float32)
        for h in range(HD):
            wvT[:, h * 65:h * 65 + 64] = wv_w[g * GW + h * 64:
                                              g * GW + (h + 1) * 64].T
            vbias[h * 65:h * 65 + 64] = wv_b[g * GW + h * 64:
                                             g * GW + (h + 1) * 64]
            vbias[h * 65 + 64] = 1.0
        # wo rows permuted: ctx chunk c, partition par*64+p0 <-> head par*4+c
        woT = np.zeros((GW, D), np.float32)
        for par in range(2):
            for c in range(4):
                h = par * 4 + c
                woT[c * 128 + par * 64:c * 128 + par * 64 + 64, :] = \
                    wo_w[:, g * GW + h * 64:g * GW + (h + 1) * 64].T
        bqT = np.ascontiguousarray((wq_b[sl] * 0.125).reshape(4, 128).T)
        bkT = np.ascontiguousarray(wk_b[sl].reshape(4, 128).T)
        vb_bc = np.broadcast_to(vbias[None, :], (128, AUGW))
        gmaps.append(dict(wq=_bf16(wqT), wk=_bf16(wkT), wv=_bf16(wvT),
                          wo=_bf16(woT), bq=bqT, bk=bkT,
                          vb=np.ascontiguousarray(vb_bc.astype(np.float32)),
                          msk=_bf16(msk)))

    bmaps = []
    for b in range(B):
        bmaps.append(dict(
            xq=_bf16(q[b].T),
            xk=_bf16(k[b].T),
            xv=_bf16(v[b].T)))

    in_maps = [dict(**bmaps[c // 2], **gmaps[c % 2]) for c in range(8)]

    trace = bool(int(os.environ.get("KERNEL_TRACE", "0")))
    res = run_bass_kernel_spmd(_NC, in_maps, list(range(8)), trace=trace)
    LAST_RESULTS = res

    out = np.empty((B, L, D), np.float32)
    for b in range(B):
        out[b] = (np.asarray(res.results[2 * b]["outp"], dtype=np.float32)
                  + np.asarray(res.results[2 * b + 1]["outp"], dtype=np.float32)
                  + wo_b[None, :])
    return out
